# revision 17
# baseline (speedup 1.0000x reference)
"""PointNet-style encoder (conv1x1 stack + ragged segment-max) on 8 Trainium2 cores.

Strategy
--------
* BN folded into the conv weights host-side; every layer becomes matmul+bias+ReLU.
* Feature-major on device: activations live as [C, points] tiles, points stream
  through the PE as the matmul free dimension in 512-point macro-tiles.
* Raggedness handled host-side: each segment's points padded to a multiple of
  512 by duplicating its own points (exact under max-pooling), so every
  macro-tile belongs to exactly one segment. Segments are bin-packed 4 per
  core (sorted smallest-first, pad tiles assigned to the last slot so the
  tile order is slot-monotone), and every core is padded to the same
  macro-tile count Tm, so a single SPMD program covers all cores.
* The mid-network segment-max feeds back via the concat identity
  concat(f2, g) @ W3 = f2 @ W3[:256] + g @ W3[256:]; the g-contribution plus
  b3 becomes a per-macro-tile bias column (table U), applied by the activation
  instruction's per-partition bias operand.
* Phase A (layers 1-2, storing f2 in fp16 + per-tile maxes) and phase B
  (layers 3-4) are emitted as ONE interleaved pipeline: host-computed static
  bounds (slot s fully processed after phase-A tile bound[s] on every core)
  let per-slot g reductions and chunked U-table production run mid-stream, so
  phase A's DVE/ACT work hides entirely under phase B's PE-bound matmuls.
* Per-macro-tile layer-4 maxes are returned raw; the host applies
  relu(. + b4) and the per-segment max over tiles (monotonicity makes this
  exact).
* Matmuls run in float16 (fp32 PSUM accumulate): 1 cycle/column, ~1e-3 rel err.
  (float32r measured 2 cycles/column on HW: it is a 2-pass replicated mode.)
"""

import numpy as np

import concourse.bass as bass
import concourse.mybir as mybir
import concourse.tile as tile
from concourse import bacc
from concourse.bass_utils import run_bass_kernel_spmd

N_CORES = 8
PT = 512  # points per macro-tile
CH = 4  # U-table production chunk (tiles)
EPS = 1e-3  # keras BatchNormalization default epsilon

F32 = mybir.dt.float32
F16 = mybir.dt.float16
AF = mybir.ActivationFunctionType
AXX = mybir.AxisListType.X
ALU_MAX = mybir.AluOpType.max

_PROGRAM_CACHE: dict = {}


def _build_program(Tm: int, S: int, bounds: tuple, s_his: tuple):
    """One SPMD program for all cores: Tm macro-tiles, S segment slots.

    bounds[s]: phase-A tile count after which slot s is complete on every core.
    s_his[k]: max slot id appearing in tile chunk k (size CH) on any core.
    """
    nc = bacc.Bacc("TRN2")
    Tmp = Tm + (Tm % 2)
    nchunks = (Tm + CH - 1) // CH

    xT = nc.dram_tensor("xT", [3, Tm * PT], F16, kind="ExternalInput")
    mask = nc.dram_tensor("mask", [128, S, Tmp], F32, kind="ExternalInput")
    w1 = nc.dram_tensor("w1", [3, 128], F16, kind="ExternalInput")
    w2 = nc.dram_tensor("w2", [128, 2, 128], F16, kind="ExternalInput")
    w3a = nc.dram_tensor("w3a", [128, 2, 4, 128], F16, kind="ExternalInput")
    w3b = nc.dram_tensor("w3b", [128, 2, 4, 128], F16, kind="ExternalInput")
    w4 = nc.dram_tensor("w4", [128, 4, 8, 128], F16, kind="ExternalInput")
    b1 = nc.dram_tensor("b1", [128, 1], F32, kind="ExternalInput")
    b2 = nc.dram_tensor("b2", [128, 2], F32, kind="ExternalInput")
    b3 = nc.dram_tensor("b3", [128, 4], F32, kind="ExternalInput")
    mx4 = nc.dram_tensor("mx4", [128, 8, Tm], F32, kind="ExternalOutput")

    with tile.TileContext(nc) as tc:
        with (
            tc.tile_pool(name="const", bufs=1) as constp,
            tc.tile_pool(name="xp", bufs=4) as xp,
            tc.tile_pool(name="h1p", bufs=3) as h1p,
            tc.tile_pool(name="h3p", bufs=3) as h3p,
            tc.tile_pool(name="tmpp", bufs=4) as tmpp,
            tc.tile_pool(name="psA", bufs=2, space="PSUM") as psA,
            tc.tile_pool(name="psB3", bufs=2, space="PSUM") as psB3,
            tc.tile_pool(name="psB4", bufs=2, space="PSUM") as psB4,
        ):
            # small, immediately-needed constants on the sync DMA queue; the
            # big phase-B weights on the gpsimd queue so they don't
            # head-of-line-block phase A's x-tile loads.
            w1_sb = constp.tile([3, 128], F16)
            nc.sync.dma_start(out=w1_sb, in_=w1.ap())
            w2_sb = constp.tile([128, 2, 128], F16)
            nc.sync.dma_start(out=w2_sb, in_=w2.ap())
            b1_sb = constp.tile([128, 1], F32)
            nc.sync.dma_start(out=b1_sb, in_=b1.ap())
            b2_sb = constp.tile([128, 2], F32)
            nc.sync.dma_start(out=b2_sb, in_=b2.ap())
            b3_sb = constp.tile([128, 4], F32)
            nc.sync.dma_start(out=b3_sb, in_=b3.ap())
            mask_sb = constp.tile([128, S, Tmp], F32)
            nc.sync.dma_start(out=mask_sb, in_=mask.ap())
            w3a_sb = constp.tile([128, 2, 4, 128], F16)
            nc.gpsimd.dma_start(out=w3a_sb, in_=w3a.ap())
            w3b_sb = constp.tile([128, 2, 4, 128], F16)
            nc.gpsimd.dma_start(out=w3b_sb, in_=w3b.ap())
            w4_sb = constp.tile([128, 4, 8, 128], F16)
            nc.gpsimd.dma_start(out=w4_sb, in_=w4.ap())

            f2_all = constp.tile([128, Tm, 2, PT], F16)
            Mx2_sb = constp.tile([128, 2, Tm], F32)
            g_sb = constp.tile([128, 2, S], F32)
            Gacc_sb = constp.tile([128, 2, Tmp], F32)
            G2_sb = constp.tile([128, 2, Tmp], F16)
            U_sb = constp.tile([128, 4, Tmp], F32)
            Mx4_sb = constp.tile([128, 8, Tm], F32)

            xTr = xT.ap()

            # HAM warmup: dependency-free matmuls keep the PE busy through the
            # DMA prologue so the clock gate opens (1.2 -> 2.4 GHz) before the
            # real matmuls start.
            warm_src = constp.tile([128, PT], F16, name="warm_src")
            nc.vector.memset(warm_src, 0.01)
            warm_out = constp.tile([128, 1], F32, name="warm_out")
            ps_w = psA.tile([128, PT], F32, tag="psa", name="ps_warm")
            for i in range(20):
                nc.tensor.matmul(
                    ps_w[:, :], warm_src[:, 0:128], warm_src[:, :], start=True, stop=True
                )
            nc.vector.tensor_reduce(out=warm_out, in_=ps_w[:, 0:8], axis=AXX, op=ALU_MAX)

            # ---------------- emission helpers ----------------
            h1s = {}

            def emit_A(t):
                """L1+L2 for tile t; stores f2 (fp16) and its per-tile max."""
                x_sb = xp.tile([3, PT], F16, tag="x", name=f"x_{t}")
                nc.sync.dma_start(out=x_sb, in_=xTr[:, t * PT : (t + 1) * PT])
                ps1 = psA.tile([128, PT], F32, tag="psa", name=f"ps1_{t}")
                nc.tensor.matmul(ps1[:, :], w1_sb[:, :], x_sb[:, :], start=True, stop=True)
                h1_sb = h1p.tile([128, PT], F16, tag="h1", name=f"h1_{t}")
                nc.scalar.activation(out=h1_sb, in_=ps1, func=AF.Relu, bias=b1_sb[:, 0:1])
                for c in range(2):
                    ps2 = psA.tile([128, PT], F32, tag="psa", name=f"ps2_{t}_{c}")
                    nc.tensor.matmul(ps2[:, :], w2_sb[:, c, :], h1_sb[:, :], start=True, stop=True)
                    if c == 0:
                        nc.scalar.activation(
                            out=f2_all[:, t, c, :], in_=ps2, func=AF.Relu, bias=b2_sb[:, c : c + 1]
                        )
                    else:
                        # relu(x + b) on the DVE to balance ACT/DVE load
                        nc.vector.tensor_scalar(
                            f2_all[:, t, c, :], ps2, b2_sb[:, c : c + 1], 0.0,
                            mybir.AluOpType.add, ALU_MAX,
                        )
                nc.vector.tensor_reduce(
                    out=Mx2_sb[:, :, t : t + 1], in_=f2_all[:, t, :, :], axis=AXX, op=ALU_MAX
                )

            def emit_g(s):
                """Per-slot max over the (host-bounded) range of Mx2 columns."""
                b = bounds[s]
                for c in range(2):
                    tmp = tmpp.tile([128, Tm], F32, tag="tmp", name=f"tmpg_{c}_{s}")
                    nc.vector.tensor_mul(tmp[:, :b], Mx2_sb[:, c, :b], mask_sb[:, s, :b])
                    nc.vector.tensor_reduce(
                        out=g_sb[:, c, s : s + 1], in_=tmp[:, :b], axis=AXX, op=ALU_MAX
                    )

            def emit_Uchunk(k):
                """U[:, :, kCH:kCH+w] = W3b.T @ G2_chunk + b3 (per-tile bias)."""
                c0 = k * CH
                w = min(CH, Tm - c0)
                we = w + (w % 2)  # keep matmul free dims even
                shi = s_his[k]
                for c in range(2):
                    nc.vector.tensor_scalar_mul(
                        Gacc_sb[:, c, c0 : c0 + we], mask_sb[:, 0, c0 : c0 + we], g_sb[:, c, 0:1]
                    )
                    for s in range(1, shi + 1):
                        tmp2 = tmpp.tile([128, CH + 1], F32, tag="tmp2", name=f"tmpe_{k}_{c}_{s}")
                        nc.vector.tensor_scalar_mul(
                            tmp2[:, :we], mask_sb[:, s, c0 : c0 + we], g_sb[:, c, s : s + 1]
                        )
                        nc.vector.tensor_add(
                            Gacc_sb[:, c, c0 : c0 + we], Gacc_sb[:, c, c0 : c0 + we], tmp2[:, :we]
                        )
                    nc.scalar.copy(G2_sb[:, c, c0 : c0 + we], Gacc_sb[:, c, c0 : c0 + we])
                for m in range(4):
                    psu = psA.tile([128, PT], F32, tag="psa", name=f"psu_{k}_{m}")
                    nc.tensor.matmul(
                        psu[:, :we], w3b_sb[:, 0, m, :], G2_sb[:, 0, c0 : c0 + we],
                        start=True, stop=False,
                    )
                    nc.tensor.matmul(
                        psu[:, :we], w3b_sb[:, 1, m, :], G2_sb[:, 1, c0 : c0 + we],
                        start=False, stop=True,
                    )
                    nc.scalar.activation(
                        out=U_sb[:, m, c0 : c0 + we], in_=psu[:, :we],
                        func=AF.Identity, bias=b3_sb[:, m : m + 1],
                    )

            h3_tiles = {}

            def emit_L3(t):
                h3_sb = h3p.tile([128, 4, PT], F16, tag="h3", name=f"h3_{t}")
                for m in range(4):
                    ps3 = psB3.tile([128, PT], F32, tag="ps3", name=f"ps3_{t}_{m}")
                    nc.tensor.matmul(
                        ps3[:, :], w3a_sb[:, 0, m, :], f2_all[:, t, 0, :], start=True, stop=False
                    )
                    nc.tensor.matmul(
                        ps3[:, :], w3a_sb[:, 1, m, :], f2_all[:, t, 1, :], start=False, stop=True
                    )
                    nc.scalar.activation(
                        out=h3_sb[:, m, :], in_=ps3, func=AF.Relu, bias=U_sb[:, m, t : t + 1]
                    )
                h3_tiles[t] = h3_sb

            def emit_L4(t):
                h3_sb = h3_tiles.pop(t)
                for mg in range(4):
                    # inner dim padded to a full PSUM bank (512 f32) so each
                    # m-chunk's matmul output stays within one bank
                    ps4 = psB4.tile([128, 2, 512], F32, tag="ps4", name=f"ps4_{t}_{mg}")
                    for mi in range(2):
                        m = mg * 2 + mi
                        for k in range(4):
                            nc.tensor.matmul(
                                ps4[:, mi, :PT], w4_sb[:, k, m, :], h3_sb[:, k, :],
                                start=(k == 0), stop=(k == 3),
                            )
                    nc.vector.tensor_reduce(
                        out=Mx4_sb[:, 2 * mg : 2 * mg + 2, t : t + 1], in_=ps4[:, :, :PT], axis=AXX, op=ALU_MAX
                    )

            # ---------------- interleaved pipeline ----------------
            a_next = 0
            b_next = 0
            l3_next = 0
            u_next = 0
            g_emitted = [False] * S

            def try_unlock():
                nonlocal u_next
                for s in range(S):
                    if not g_emitted[s] and a_next >= bounds[s]:
                        emit_g(s)
                        g_emitted[s] = True
                while u_next < nchunks and all(g_emitted[s] for s in range(s_his[u_next] + 1)):
                    emit_Uchunk(u_next)
                    u_next += 1

            # phase A must lead phase B by enough tiles that B's U-table
            # chunks are always unlocked when its L3s reach the PE queue
            need = [bounds[s_his[min(i + 1, Tm - 1) // CH]] for i in range(Tm)]
            LEAD = max(max(need[i] - i for i in range(Tm)) + 1, need[0])

            while b_next < Tm:
                while a_next < min(Tm, b_next + LEAD):
                    emit_A(a_next)
                    a_next += 1
                    try_unlock()
                progressed = False
                while (
                    l3_next <= min(b_next + 1, Tm - 1)
                    and l3_next // CH < u_next
                    and l3_next < a_next
                ):
                    emit_L3(l3_next)
                    l3_next += 1
                    progressed = True
                if l3_next > b_next:
                    emit_L4(b_next)
                    b_next += 1
                    progressed = True
                if not progressed:
                    if a_next < Tm:
                        emit_A(a_next)
                        a_next += 1
                        try_unlock()
                    else:
                        raise RuntimeError("pipeline deadlock")

            nc.sync.dma_start(out=mx4.ap(), in_=Mx4_sb)

    nc.finalize()
    return nc


def _prepare(x, seg_ids, B):
    """Pad + pack segments into per-core, slot-monotone macro-tile streams."""
    counts = np.bincount(seg_ids, minlength=B)
    starts = np.concatenate([[0], np.cumsum(counts)])
    seg_tiles = [(int(c) + PT - 1) // PT for c in counts]

    SLOTS = (B + N_CORES - 1) // N_CORES
    order = np.argsort(-np.asarray(seg_tiles), kind="stable")
    core_segs: list[list[int]] = [[] for _ in range(N_CORES)]
    core_load = [0] * N_CORES
    for s in order:
        cands = [c for c in range(N_CORES) if len(core_segs[c]) < SLOTS]
        c = min(cands, key=lambda i: core_load[i])
        core_segs[c].append(int(s))
        core_load[c] += seg_tiles[s]

    # local search: swap segments between cores to shave the max load
    ideal = (sum(seg_tiles) + N_CORES - 1) // N_CORES
    for _ in range(200):
        if max(core_load) <= ideal:
            break
        hi = max(range(N_CORES), key=lambda i: core_load[i])
        improved = False
        for lo in sorted(range(N_CORES), key=lambda i: core_load[i]):
            if lo == hi:
                continue
            for ia, sa in enumerate(core_segs[hi]):
                for ib, sb in enumerate(core_segs[lo]):
                    d = seg_tiles[sa] - seg_tiles[sb]
                    if d > 0 and max(core_load[hi] - d, core_load[lo] + d) < max(
                        core_load[hi], core_load[lo]
                    ):
                        core_segs[hi][ia], core_segs[lo][ib] = sb, sa
                        core_load[hi] -= d
                        core_load[lo] += d
                        improved = True
                        break
                if improved:
                    break
            if improved:
                break
        if not improved:
            break
    Tm = max(core_load)

    # order each core's slots so cumulative tile counts track uniform
    # targets across cores => tight static pipeline bounds (small LEAD)
    from itertools import permutations

    targets = [Tm * (i + 1) / SLOTS for i in range(SLOTS)]
    for c in range(N_CORES):
        best, best_score = None, None
        for perm in permutations(core_segs[c]):
            cs, score = 0, 0.0
            for i, s in enumerate(perm):
                cs += seg_tiles[s]
                score += abs(cs - targets[i])
            if best_score is None or score < best_score:
                best, best_score = perm, score
        core_segs[c] = list(best)

    xT_cores, mask_cores, post = [], [], []
    sots = []
    for c in range(N_CORES):
        pts_list, slot_of_tile = [], []
        for slot, s in enumerate(core_segs[c]):
            seg_pts = x[starts[s] : starts[s + 1]]
            ntile = seg_tiles[s]
            padn = ntile * PT - len(seg_pts)
            if padn:
                seg_pts = np.concatenate([seg_pts, seg_pts[:padn]])
            pts_list.append(seg_pts)
            slot_of_tile += [slot] * ntile
        extra = Tm - core_load[c]
        if extra:
            # core-equalization pad tiles duplicate the LAST slot's points so
            # the tile order stays slot-monotone
            pts_list.append(np.tile(pts_list[-1][:PT], (extra, 1)))
            slot_of_tile += [SLOTS - 1] * extra
        xc = np.concatenate(pts_list).astype(np.float16)
        xT_cores.append(np.ascontiguousarray(xc.T))
        sot = np.asarray(slot_of_tile)
        sots.append(sot)
        Tmp = Tm + (Tm % 2)
        m01 = np.zeros((SLOTS, Tmp), np.float32)
        m01[:, :Tm] = sot[None, :] == np.arange(SLOTS)[:, None]
        mask_cores.append(np.ascontiguousarray(np.broadcast_to(m01[None], (128, SLOTS, Tmp))))
        post.append((core_segs[c], sot))

    # static pipeline bounds (shared across cores)
    bounds = tuple(
        int(max(np.flatnonzero(sot == s).max() for sot in sots)) + 1 for s in range(SLOTS)
    )
    nchunks = (Tm + CH - 1) // CH
    s_his = tuple(
        int(max(sot[k * CH : min((k + 1) * CH, Tm)].max() for sot in sots))
        for k in range(nchunks)
    )
    return Tm, SLOTS, xT_cores, mask_cores, post, bounds, s_his


def make_in_maps(inputs):
    """Fold BN, pack points, and build the per-core SPMD input dicts.

    Returns (key, in_maps, post, b4f) where key indexes _PROGRAM_CACHE.
    """
    x = np.asarray(inputs["x"], np.float32)
    seg_ids = np.asarray(inputs["seg_ids"])
    B = int(inputs["num_segments"])

    Wf, bf = [], []
    for i in (1, 2, 3, 4):
        W = np.asarray(inputs[f"W{i}"], np.float32)
        b = np.asarray(inputs[f"b{i}"], np.float32)
        ga = np.asarray(inputs[f"g{i}"], np.float32)
        be = np.asarray(inputs[f"be{i}"], np.float32)
        m = np.asarray(inputs[f"m{i}"], np.float32)
        v = np.asarray(inputs[f"v{i}"], np.float32)
        sc = ga / np.sqrt(v + EPS)
        Wf.append(np.ascontiguousarray(W * sc[None, :]))
        bf.append((b - m) * sc + be)
    W1f, W2f, W3f, W4f = Wf
    b1f, b2f, b3f, b4f = bf

    Tm, SLOTS, xT_cores, mask_cores, post, bounds, s_his = _prepare(x, seg_ids, B)

    w1d = W1f.astype(np.float16)
    w2d = np.ascontiguousarray(W2f.reshape(128, 2, 128).astype(np.float16))
    w3ad = np.ascontiguousarray(W3f[:256].reshape(2, 128, 4, 128).transpose(1, 0, 2, 3).astype(np.float16))
    w3bd = np.ascontiguousarray(W3f[256:].reshape(2, 128, 4, 128).transpose(1, 0, 2, 3).astype(np.float16))
    w4d = np.ascontiguousarray(W4f.reshape(4, 128, 8, 128).transpose(1, 0, 2, 3).astype(np.float16))
    b1d = np.ascontiguousarray(b1f.reshape(128, 1))
    b2d = np.ascontiguousarray(b2f.reshape(2, 128).T)
    b3d = np.ascontiguousarray(b3f.reshape(4, 128).T)

    in_maps = [
        {
            "xT": xT_cores[c],
            "mask": mask_cores[c],
            "w1": w1d,
            "w2": w2d,
            "w3a": w3ad,
            "w3b": w3bd,
            "w4": w4d,
            "b1": b1d,
            "b2": b2d,
            "b3": b3d,
        }
        for c in range(N_CORES)
    ]
    return (Tm, SLOTS, bounds, s_his), in_maps, post, b4f


def postprocess(results, post, b4f, B):
    out = np.zeros((B, 1024), np.float32)
    for c in range(N_CORES):
        mx4 = results[c]["mx4"]  # [128, 8, Tm]
        segs, sot = post[c]
        for slot, s in enumerate(segs):
            cols = np.flatnonzero(sot == slot)
            raw = mx4[:, :, cols].max(axis=2)  # [128, 8]
            out[s] = np.maximum(raw.T.reshape(1024) + b4f, 0.0)
    return out


def get_program(key):
    if key not in _PROGRAM_CACHE:
        _PROGRAM_CACHE[key] = _build_program(*key)
    return _PROGRAM_CACHE[key]


def kernel(**inputs) -> np.ndarray:
    B = int(inputs["num_segments"])
    key, in_maps, post, b4f = make_in_maps(inputs)
    nc = get_program(key)
    res = run_bass_kernel_spmd(nc, in_maps, core_ids=list(range(N_CORES)))
    return postprocess(res.results, post, b4f, B)


# revision 18
# speedup vs baseline: 1.0026x; 1.0026x over previous
"""PointNet-style encoder (conv1x1 stack + ragged segment-max) on 8 Trainium2 cores.

Strategy
--------
* BN folded into the conv weights host-side; every layer becomes matmul+bias+ReLU.
* Feature-major on device: activations live as [C, points] tiles, points stream
  through the PE as the matmul free dimension in 512-point macro-tiles.
* Raggedness handled host-side: each segment's points padded to a multiple of
  512 by duplicating its own points (exact under max-pooling), so every
  macro-tile belongs to exactly one segment. Segments are bin-packed 4 per
  core (sorted smallest-first, pad tiles assigned to the last slot so the
  tile order is slot-monotone), and every core is padded to the same
  macro-tile count Tm, so a single SPMD program covers all cores.
* The mid-network segment-max feeds back via the concat identity
  concat(f2, g) @ W3 = f2 @ W3[:256] + g @ W3[256:]; the g-contribution plus
  b3 becomes a per-macro-tile bias column (table U), applied by the activation
  instruction's per-partition bias operand.
* Phase A (layers 1-2, storing f2 in fp16 + per-tile maxes) and phase B
  (layers 3-4) are emitted as ONE interleaved pipeline: host-computed static
  bounds (slot s fully processed after phase-A tile bound[s] on every core)
  let per-slot g reductions and chunked U-table production run mid-stream, so
  phase A's DVE/ACT work hides entirely under phase B's PE-bound matmuls.
* Per-macro-tile layer-4 maxes are returned raw; the host applies
  relu(. + b4) and the per-segment max over tiles (monotonicity makes this
  exact).
* Matmuls run in float16 (fp32 PSUM accumulate): 1 cycle/column, ~1e-3 rel err.
  (float32r measured 2 cycles/column on HW: it is a 2-pass replicated mode.)
"""

import numpy as np

import concourse.bass as bass
import concourse.mybir as mybir
import concourse.tile as tile
from concourse import bacc
from concourse.bass_utils import run_bass_kernel_spmd

N_CORES = 8
PT = 512  # points per macro-tile
CH = 4  # U-table production chunk (tiles)
EPS = 1e-3  # keras BatchNormalization default epsilon

F32 = mybir.dt.float32
F16 = mybir.dt.float16
AF = mybir.ActivationFunctionType
AXX = mybir.AxisListType.X
ALU_MAX = mybir.AluOpType.max

_PROGRAM_CACHE: dict = {}


def _build_program(Tm: int, S: int, bounds: tuple, s_his: tuple):
    """One SPMD program for all cores: Tm macro-tiles, S segment slots.

    bounds[s]: phase-A tile count after which slot s is complete on every core.
    s_his[k]: max slot id appearing in tile chunk k (size CH) on any core.
    """
    nc = bacc.Bacc("TRN2")
    Tmp = Tm + (Tm % 2)
    nchunks = (Tm + CH - 1) // CH

    xT = nc.dram_tensor("xT", [3, Tm * PT], F16, kind="ExternalInput")
    mask = nc.dram_tensor("mask", [128, S, Tmp], F32, kind="ExternalInput")
    w1 = nc.dram_tensor("w1", [3, 128], F16, kind="ExternalInput")
    w2 = nc.dram_tensor("w2", [128, 2, 128], F16, kind="ExternalInput")
    w3a = nc.dram_tensor("w3a", [128, 2, 4, 128], F16, kind="ExternalInput")
    w3b = nc.dram_tensor("w3b", [128, 2, 4, 128], F16, kind="ExternalInput")
    w4 = nc.dram_tensor("w4", [128, 4, 8, 128], F16, kind="ExternalInput")
    b1 = nc.dram_tensor("b1", [128, 1], F32, kind="ExternalInput")
    b2 = nc.dram_tensor("b2", [128, 2], F32, kind="ExternalInput")
    b3 = nc.dram_tensor("b3", [128, 4], F32, kind="ExternalInput")
    mx4 = nc.dram_tensor("mx4", [128, 8, Tm], F32, kind="ExternalOutput")

    with tile.TileContext(nc) as tc:
        with (
            tc.tile_pool(name="const", bufs=1) as constp,
            tc.tile_pool(name="xp", bufs=4) as xp,
            tc.tile_pool(name="h1p", bufs=3) as h1p,
            tc.tile_pool(name="h3p", bufs=3) as h3p,
            tc.tile_pool(name="tmpp", bufs=4) as tmpp,
            tc.tile_pool(name="psA", bufs=2, space="PSUM") as psA,
            tc.tile_pool(name="psB3", bufs=2, space="PSUM") as psB3,
            tc.tile_pool(name="psB4", bufs=2, space="PSUM") as psB4,
        ):
            # small, immediately-needed constants on the sync DMA queue; the
            # big phase-B weights on the gpsimd queue so they don't
            # head-of-line-block phase A's x-tile loads.
            w1_sb = constp.tile([3, 128], F16)
            nc.sync.dma_start(out=w1_sb, in_=w1.ap())
            w2_sb = constp.tile([128, 2, 128], F16)
            nc.sync.dma_start(out=w2_sb, in_=w2.ap())
            b1_sb = constp.tile([128, 1], F32)
            nc.sync.dma_start(out=b1_sb, in_=b1.ap())
            b2_sb = constp.tile([128, 2], F32)
            nc.sync.dma_start(out=b2_sb, in_=b2.ap())
            b3_sb = constp.tile([128, 4], F32)
            nc.sync.dma_start(out=b3_sb, in_=b3.ap())
            mask_sb = constp.tile([128, S, Tmp], F32)
            nc.sync.dma_start(out=mask_sb, in_=mask.ap())
            w3a_sb = constp.tile([128, 2, 4, 128], F16)
            nc.gpsimd.dma_start(out=w3a_sb, in_=w3a.ap())
            w3b_sb = constp.tile([128, 2, 4, 128], F16)
            nc.gpsimd.dma_start(out=w3b_sb, in_=w3b.ap())
            w4_sb = constp.tile([128, 4, 8, 128], F16)
            nc.gpsimd.dma_start(out=w4_sb, in_=w4.ap())

            f2_all = constp.tile([128, Tm, 2, PT], F16)
            Mx2_sb = constp.tile([128, 2, Tm], F32)
            g_sb = constp.tile([128, 2, S], F32)
            Gacc_sb = constp.tile([128, 2, Tmp], F32)
            G2_sb = constp.tile([128, 2, Tmp], F16)
            U_sb = constp.tile([128, 4, Tmp], F32)
            Mx4_sb = constp.tile([128, 8, Tm], F32)

            xTr = xT.ap()

            # HAM warmup: dependency-free matmuls keep the PE busy through the
            # DMA prologue so the clock gate opens (1.2 -> 2.4 GHz) before the
            # real matmuls start.
            warm_src = constp.tile([128, PT], F16, name="warm_src")
            nc.vector.memset(warm_src, 0.01)
            warm_out = constp.tile([128, 1], F32, name="warm_out")
            ps_w = psA.tile([128, PT], F32, tag="psa", name="ps_warm")
            for i in range(20):
                nc.tensor.matmul(
                    ps_w[:, :], warm_src[:, 0:128], warm_src[:, :], start=True, stop=True
                )
            nc.vector.tensor_reduce(out=warm_out, in_=ps_w[:, 0:8], axis=AXX, op=ALU_MAX)

            # ---------------- emission helpers ----------------
            h1s = {}

            def emit_A(t):
                """L1+L2 for tile t; stores f2 (fp16) and its per-tile max."""
                x_sb = xp.tile([3, PT], F16, tag="x", name=f"x_{t}")
                nc.sync.dma_start(out=x_sb, in_=xTr[:, t * PT : (t + 1) * PT])
                ps1 = psA.tile([128, PT], F32, tag="psa", name=f"ps1_{t}")
                nc.tensor.matmul(ps1[:, :], w1_sb[:, :], x_sb[:, :], start=True, stop=True)
                h1_sb = h1p.tile([128, PT], F16, tag="h1", name=f"h1_{t}")
                nc.scalar.activation(out=h1_sb, in_=ps1, func=AF.Relu, bias=b1_sb[:, 0:1])
                for c in range(2):
                    ps2 = psA.tile([128, PT], F32, tag="psa", name=f"ps2_{t}_{c}")
                    nc.tensor.matmul(ps2[:, :], w2_sb[:, c, :], h1_sb[:, :], start=True, stop=True)
                    if c == 0:
                        nc.scalar.activation(
                            out=f2_all[:, t, c, :], in_=ps2, func=AF.Relu, bias=b2_sb[:, c : c + 1]
                        )
                    else:
                        # relu(x + b) on the DVE to balance ACT/DVE load
                        nc.vector.tensor_scalar(
                            f2_all[:, t, c, :], ps2, b2_sb[:, c : c + 1], 0.0,
                            mybir.AluOpType.add, ALU_MAX,
                        )
                nc.vector.tensor_reduce(
                    out=Mx2_sb[:, :, t : t + 1], in_=f2_all[:, t, :, :], axis=AXX, op=ALU_MAX
                )

            def emit_g(s):
                """Per-slot max over the (host-bounded) range of Mx2 columns."""
                b = bounds[s]
                for c in range(2):
                    tmp = tmpp.tile([128, Tm], F32, tag="tmp", name=f"tmpg_{c}_{s}")
                    nc.vector.tensor_mul(tmp[:, :b], Mx2_sb[:, c, :b], mask_sb[:, s, :b])
                    nc.vector.tensor_reduce(
                        out=g_sb[:, c, s : s + 1], in_=tmp[:, :b], axis=AXX, op=ALU_MAX
                    )

            def emit_Uchunk(k):
                """U[:, :, kCH:kCH+w] = W3b.T @ G2_chunk + b3 (per-tile bias)."""
                c0 = k * CH
                w = min(CH, Tm - c0)
                we = w + (w % 2)  # keep matmul free dims even
                shi = s_his[k]
                for c in range(2):
                    nc.vector.tensor_scalar_mul(
                        Gacc_sb[:, c, c0 : c0 + we], mask_sb[:, 0, c0 : c0 + we], g_sb[:, c, 0:1]
                    )
                    for s in range(1, shi + 1):
                        tmp2 = tmpp.tile([128, CH + 1], F32, tag="tmp2", name=f"tmpe_{k}_{c}_{s}")
                        nc.vector.tensor_scalar_mul(
                            tmp2[:, :we], mask_sb[:, s, c0 : c0 + we], g_sb[:, c, s : s + 1]
                        )
                        nc.vector.tensor_add(
                            Gacc_sb[:, c, c0 : c0 + we], Gacc_sb[:, c, c0 : c0 + we], tmp2[:, :we]
                        )
                    nc.scalar.copy(G2_sb[:, c, c0 : c0 + we], Gacc_sb[:, c, c0 : c0 + we])
                for m in range(4):
                    psu = psA.tile([128, PT], F32, tag="psa", name=f"psu_{k}_{m}")
                    nc.tensor.matmul(
                        psu[:, :we], w3b_sb[:, 0, m, :], G2_sb[:, 0, c0 : c0 + we],
                        start=True, stop=False,
                    )
                    nc.tensor.matmul(
                        psu[:, :we], w3b_sb[:, 1, m, :], G2_sb[:, 1, c0 : c0 + we],
                        start=False, stop=True,
                    )
                    nc.scalar.activation(
                        out=U_sb[:, m, c0 : c0 + we], in_=psu[:, :we],
                        func=AF.Identity, bias=b3_sb[:, m : m + 1],
                    )

            h3_tiles = {}

            def emit_L3(t):
                h3_sb = h3p.tile([128, 4, PT], F16, tag="h3", name=f"h3_{t}")
                for m in range(4):
                    ps3 = psB3.tile([128, PT], F32, tag="ps3", name=f"ps3_{t}_{m}")
                    nc.tensor.matmul(
                        ps3[:, :], w3a_sb[:, 0, m, :], f2_all[:, t, 0, :], start=True, stop=False
                    )
                    nc.tensor.matmul(
                        ps3[:, :], w3a_sb[:, 1, m, :], f2_all[:, t, 1, :], start=False, stop=True
                    )
                    nc.scalar.activation(
                        out=h3_sb[:, m, :], in_=ps3, func=AF.Relu, bias=U_sb[:, m, t : t + 1]
                    )
                h3_tiles[t] = h3_sb

            def emit_L4(t):
                h3_sb = h3_tiles.pop(t)
                for mg in range(4):
                    # inner dim padded to a full PSUM bank (512 f32) so each
                    # m-chunk's matmul output stays within one bank
                    ps4 = psB4.tile([128, 2, 512], F32, tag="ps4", name=f"ps4_{t}_{mg}")
                    for mi in range(2):
                        m = mg * 2 + mi
                        for k in range(4):
                            nc.tensor.matmul(
                                ps4[:, mi, :PT], w4_sb[:, k, m, :], h3_sb[:, k, :],
                                start=(k == 0), stop=(k == 3),
                            )
                    nc.vector.tensor_reduce(
                        out=Mx4_sb[:, 2 * mg : 2 * mg + 2, t : t + 1], in_=ps4[:, :, :PT], axis=AXX, op=ALU_MAX
                    )

            # ---------------- interleaved pipeline ----------------
            a_next = 0
            b_next = 0
            l3_next = 0
            u_next = 0
            g_emitted = [False] * S

            def try_unlock():
                nonlocal u_next
                for s in range(S):
                    if not g_emitted[s] and a_next >= bounds[s]:
                        emit_g(s)
                        g_emitted[s] = True
                while u_next < nchunks and all(g_emitted[s] for s in range(s_his[u_next] + 1)):
                    emit_Uchunk(u_next)
                    u_next += 1

            # phase A must lead phase B by enough tiles that B's U-table
            # chunks are always unlocked when its L3s reach the PE queue
            need = [bounds[s_his[min(i + 1, Tm - 1) // CH]] for i in range(Tm)]
            LEAD = max(max(need[i] - i for i in range(Tm)) + 1, need[0])

            while b_next < Tm:
                while a_next < min(Tm, b_next + LEAD):
                    emit_A(a_next)
                    a_next += 1
                    try_unlock()
                progressed = False
                while (
                    l3_next <= min(b_next + 1, Tm - 1)
                    and l3_next // CH < u_next
                    and l3_next < a_next
                ):
                    emit_L3(l3_next)
                    l3_next += 1
                    progressed = True
                if l3_next > b_next:
                    emit_L4(b_next)
                    b_next += 1
                    progressed = True
                if not progressed:
                    if a_next < Tm:
                        emit_A(a_next)
                        a_next += 1
                        try_unlock()
                    else:
                        raise RuntimeError("pipeline deadlock")

            nc.sync.dma_start(out=mx4.ap(), in_=Mx4_sb)

    nc.finalize()
    return nc


def _prepare(x, seg_ids, B):
    """Pad + pack segments into per-core, slot-monotone macro-tile streams."""
    counts = np.bincount(seg_ids, minlength=B)
    starts = np.concatenate([[0], np.cumsum(counts)])
    seg_tiles = [(int(c) + PT - 1) // PT for c in counts]

    SLOTS = (B + N_CORES - 1) // N_CORES
    order = np.argsort(-np.asarray(seg_tiles), kind="stable")
    core_segs: list[list[int]] = [[] for _ in range(N_CORES)]
    core_load = [0] * N_CORES
    for s in order:
        cands = [c for c in range(N_CORES) if len(core_segs[c]) < SLOTS]
        c = min(cands, key=lambda i: core_load[i])
        core_segs[c].append(int(s))
        core_load[c] += seg_tiles[s]

    # local search: swap segments between cores to shave the max load
    ideal = (sum(seg_tiles) + N_CORES - 1) // N_CORES
    for _ in range(200):
        if max(core_load) <= ideal:
            break
        hi = max(range(N_CORES), key=lambda i: core_load[i])
        improved = False
        for lo in sorted(range(N_CORES), key=lambda i: core_load[i]):
            if lo == hi:
                continue
            for ia, sa in enumerate(core_segs[hi]):
                for ib, sb in enumerate(core_segs[lo]):
                    d = seg_tiles[sa] - seg_tiles[sb]
                    if d > 0 and max(core_load[hi] - d, core_load[lo] + d) < max(
                        core_load[hi], core_load[lo]
                    ):
                        core_segs[hi][ia], core_segs[lo][ib] = sb, sa
                        core_load[hi] -= d
                        core_load[lo] += d
                        improved = True
                        break
                if improved:
                    break
            if improved:
                break
        if not improved:
            break
    Tm = max(core_load)

    # order each core's slots so cumulative tile counts track uniform
    # targets across cores => tight static pipeline bounds (small LEAD)
    from itertools import permutations

    targets = [Tm * (i + 1) / SLOTS for i in range(SLOTS)]
    for c in range(N_CORES):
        best, best_score = None, None
        for perm in permutations(core_segs[c]):
            cs, score = 0, 0.0
            for i, s in enumerate(perm):
                cs += seg_tiles[s]
                score += abs(cs - targets[i])
            if best_score is None or score < best_score:
                best, best_score = perm, score
        core_segs[c] = list(best)

    xT_cores, mask_cores, post = [], [], []
    sots = []
    for c in range(N_CORES):
        pts_list, slot_of_tile = [], []
        for slot, s in enumerate(core_segs[c]):
            seg_pts = x[starts[s] : starts[s + 1]]
            ntile = seg_tiles[s]
            padn = ntile * PT - len(seg_pts)
            if padn:
                seg_pts = np.concatenate([seg_pts, seg_pts[:padn]])
            pts_list.append(seg_pts)
            slot_of_tile += [slot] * ntile
        extra = Tm - core_load[c]
        if extra:
            # core-equalization pad tiles duplicate the LAST slot's points so
            # the tile order stays slot-monotone
            pts_list.append(np.tile(pts_list[-1][:PT], (extra, 1)))
            slot_of_tile += [SLOTS - 1] * extra
        xc = np.concatenate(pts_list).astype(np.float16)
        xT_cores.append(np.ascontiguousarray(xc.T))
        sot = np.asarray(slot_of_tile)
        sots.append(sot)
        Tmp = Tm + (Tm % 2)
        m01 = np.zeros((SLOTS, Tmp), np.float32)
        m01[:, :Tm] = sot[None, :] == np.arange(SLOTS)[:, None]
        mask_cores.append(np.ascontiguousarray(np.broadcast_to(m01[None], (128, SLOTS, Tmp))))
        post.append((core_segs[c], sot))

    # static pipeline bounds (shared across cores)
    bounds = tuple(
        int(max(np.flatnonzero(sot == s).max() for sot in sots)) + 1 for s in range(SLOTS)
    )
    nchunks = (Tm + CH - 1) // CH
    s_his = tuple(
        int(max(sot[k * CH : min((k + 1) * CH, Tm)].max() for sot in sots))
        for k in range(nchunks)
    )
    return Tm, SLOTS, xT_cores, mask_cores, post, bounds, s_his


def make_in_maps(inputs):
    """Fold BN, pack points, and build the per-core SPMD input dicts.

    Returns (key, in_maps, post, b4f) where key indexes _PROGRAM_CACHE.
    """
    x = np.asarray(inputs["x"], np.float32)
    seg_ids = np.asarray(inputs["seg_ids"])
    B = int(inputs["num_segments"])

    Wf, bf = [], []
    for i in (1, 2, 3, 4):
        W = np.asarray(inputs[f"W{i}"], np.float32)
        b = np.asarray(inputs[f"b{i}"], np.float32)
        ga = np.asarray(inputs[f"g{i}"], np.float32)
        be = np.asarray(inputs[f"be{i}"], np.float32)
        m = np.asarray(inputs[f"m{i}"], np.float32)
        v = np.asarray(inputs[f"v{i}"], np.float32)
        sc = ga / np.sqrt(v + EPS)
        Wf.append(np.ascontiguousarray(W * sc[None, :]))
        bf.append((b - m) * sc + be)
    W1f, W2f, W3f, W4f = Wf
    b1f, b2f, b3f, b4f = bf

    Tm, SLOTS, xT_cores, mask_cores, post, bounds, s_his = _prepare(x, seg_ids, B)

    w1d = W1f.astype(np.float16)
    w2d = np.ascontiguousarray(W2f.reshape(128, 2, 128).astype(np.float16))
    w3ad = np.ascontiguousarray(W3f[:256].reshape(2, 128, 4, 128).transpose(1, 0, 2, 3).astype(np.float16))
    w3bd = np.ascontiguousarray(W3f[256:].reshape(2, 128, 4, 128).transpose(1, 0, 2, 3).astype(np.float16))
    w4d = np.ascontiguousarray(W4f.reshape(4, 128, 8, 128).transpose(1, 0, 2, 3).astype(np.float16))
    b1d = np.ascontiguousarray(b1f.reshape(128, 1))
    b2d = np.ascontiguousarray(b2f.reshape(2, 128).T)
    b3d = np.ascontiguousarray(b3f.reshape(4, 128).T)

    in_maps = [
        {
            "xT": xT_cores[c],
            "mask": mask_cores[c],
            "w1": w1d,
            "w2": w2d,
            "w3a": w3ad,
            "w3b": w3bd,
            "w4": w4d,
            "b1": b1d,
            "b2": b2d,
            "b3": b3d,
        }
        for c in range(N_CORES)
    ]
    return (Tm, SLOTS, bounds, s_his), in_maps, post, b4f


def postprocess(results, post, b4f, B):
    out = np.zeros((B, 1024), np.float32)
    for c in range(N_CORES):
        mx4 = results[c]["mx4"]  # [128, 8, Tm]
        segs, sot = post[c]
        for slot, s in enumerate(segs):
            cols = np.flatnonzero(sot == slot)
            raw = mx4[:, :, cols].max(axis=2)  # [128, 8]
            out[s] = np.maximum(raw.T.reshape(1024) + b4f, 0.0)
    return out


def get_program(key):
    if key not in _PROGRAM_CACHE:
        _PROGRAM_CACHE[key] = _build_program(*key)
    return _PROGRAM_CACHE[key]


def kernel(**inputs) -> np.ndarray:
    B = int(inputs["num_segments"])
    key, in_maps, post, b4f = make_in_maps(inputs)
    nc = get_program(key)
    last_err = None
    for _ in range(3):  # retry transient NRT device wedges
        try:
            res = run_bass_kernel_spmd(nc, in_maps, core_ids=list(range(N_CORES)))
            return postprocess(res.results, post, b4f, B)
        except Exception as e:  # noqa: BLE001
            last_err = e
    raise last_err


# revision 20
# speedup vs baseline: 1.0094x; 1.0068x over previous
"""PointNet-style encoder (conv1x1 stack + ragged segment-max) on 8 Trainium2 cores.

Strategy
--------
* BN folded into the conv weights host-side; every layer becomes matmul+bias+ReLU.
* Feature-major on device: activations live as [C, points] tiles, points stream
  through the PE as the matmul free dimension in 512-point macro-tiles.
* Raggedness handled host-side: each segment's points padded to a multiple of
  512 by duplicating its own points (exact under max-pooling), so every
  macro-tile belongs to exactly one segment. Segments are bin-packed 4 per
  core (sorted smallest-first, pad tiles assigned to the last slot so the
  tile order is slot-monotone), and every core is padded to the same
  macro-tile count Tm, so a single SPMD program covers all cores.
* The mid-network segment-max feeds back via the concat identity
  concat(f2, g) @ W3 = f2 @ W3[:256] + g @ W3[256:]; the g-contribution plus
  b3 becomes a per-macro-tile bias column (table U), applied by the activation
  instruction's per-partition bias operand.
* Phase A (layers 1-2, storing f2 in fp16 + per-tile maxes) and phase B
  (layers 3-4) are emitted as ONE interleaved pipeline: host-computed static
  bounds (slot s fully processed after phase-A tile bound[s] on every core)
  let per-slot g reductions and chunked U-table production run mid-stream, so
  phase A's DVE/ACT work hides entirely under phase B's PE-bound matmuls.
* Per-macro-tile layer-4 maxes are returned raw; the host applies
  relu(. + b4) and the per-segment max over tiles (monotonicity makes this
  exact).
* Matmuls run in float16 (fp32 PSUM accumulate): 1 cycle/column, ~1e-3 rel err.
  (float32r measured 2 cycles/column on HW: it is a 2-pass replicated mode.)
"""

import numpy as np

import concourse.bass as bass
import concourse.mybir as mybir
import concourse.tile as tile
from concourse import bacc
from concourse.bass_utils import run_bass_kernel_spmd

N_CORES = 8
PT = 512  # points per macro-tile
CH = 4  # U-table production chunk (tiles)
EPS = 1e-3  # keras BatchNormalization default epsilon

F32 = mybir.dt.float32
F16 = mybir.dt.float16
AF = mybir.ActivationFunctionType
AXX = mybir.AxisListType.X
ALU_MAX = mybir.AluOpType.max

_PROGRAM_CACHE: dict = {}


def _build_program(Tm: int, S: int, bounds: tuple, s_his: tuple):
    """One SPMD program for all cores: Tm macro-tiles, S segment slots.

    bounds[s]: phase-A tile count after which slot s is complete on every core.
    s_his[k]: max slot id appearing in tile chunk k (size CH) on any core.
    """
    nc = bacc.Bacc("TRN2")
    Tmp = Tm + (Tm % 2)
    nchunks = (Tm + CH - 1) // CH

    xT = nc.dram_tensor("xT", [3, Tm * PT], F16, kind="ExternalInput")
    mask = nc.dram_tensor("mask", [128, S, Tmp], F32, kind="ExternalInput")
    w1 = nc.dram_tensor("w1", [3, 128], F16, kind="ExternalInput")
    w2 = nc.dram_tensor("w2", [128, 2, 128], F16, kind="ExternalInput")
    w3a = nc.dram_tensor("w3a", [128, 2, 4, 128], F16, kind="ExternalInput")
    w3b = nc.dram_tensor("w3b", [128, 2, 4, 128], F16, kind="ExternalInput")
    w4 = nc.dram_tensor("w4", [128, 4, 8, 128], F16, kind="ExternalInput")
    b1 = nc.dram_tensor("b1", [128, 1], F32, kind="ExternalInput")
    b2 = nc.dram_tensor("b2", [128, 2], F32, kind="ExternalInput")
    b3 = nc.dram_tensor("b3", [128, 4], F32, kind="ExternalInput")
    mx4 = nc.dram_tensor("mx4", [128, 8, Tm], F32, kind="ExternalOutput")

    with tile.TileContext(nc) as tc:
        with (
            tc.tile_pool(name="const", bufs=1) as constp,
            tc.tile_pool(name="xp", bufs=4) as xp,
            tc.tile_pool(name="h1p", bufs=3) as h1p,
            tc.tile_pool(name="h3p", bufs=3) as h3p,
            tc.tile_pool(name="tmpp", bufs=4) as tmpp,
            tc.tile_pool(name="psA", bufs=2, space="PSUM") as psA,
            tc.tile_pool(name="psB3", bufs=2, space="PSUM") as psB3,
            tc.tile_pool(name="psB4", bufs=2, space="PSUM") as psB4,
        ):
            # small, immediately-needed constants on the sync DMA queue; the
            # big phase-B weights on the gpsimd queue so they don't
            # head-of-line-block phase A's x-tile loads.
            w1_sb = constp.tile([3, 128], F16)
            nc.sync.dma_start(out=w1_sb, in_=w1.ap())
            w2_sb = constp.tile([128, 2, 128], F16)
            nc.sync.dma_start(out=w2_sb, in_=w2.ap())
            b1_sb = constp.tile([128, 1], F32)
            nc.sync.dma_start(out=b1_sb, in_=b1.ap())
            b2_sb = constp.tile([128, 2], F32)
            nc.sync.dma_start(out=b2_sb, in_=b2.ap())
            b3_sb = constp.tile([128, 4], F32)
            nc.sync.dma_start(out=b3_sb, in_=b3.ap())
            mask_sb = constp.tile([128, S, Tmp], F32)
            nc.sync.dma_start(out=mask_sb, in_=mask.ap())
            w3a_sb = constp.tile([128, 2, 4, 128], F16)
            nc.gpsimd.dma_start(out=w3a_sb, in_=w3a.ap())
            w3b_sb = constp.tile([128, 2, 4, 128], F16)
            nc.gpsimd.dma_start(out=w3b_sb, in_=w3b.ap())
            w4_sb = constp.tile([128, 4, 8, 128], F16)
            nc.gpsimd.dma_start(out=w4_sb, in_=w4.ap())

            f2_all = constp.tile([128, Tm, 2, PT], F16)
            Mx2_sb = constp.tile([128, 2, Tm], F32)
            g_sb = constp.tile([128, 2, S], F32)
            Gacc_sb = constp.tile([128, 2, Tmp], F32)
            G2_sb = constp.tile([128, 2, Tmp], F16)
            U_sb = constp.tile([128, 4, Tmp], F32)
            Mx4_sb = constp.tile([128, 8, Tm], F32)

            xTr = xT.ap()

            # HAM warmup: dependency-free matmuls keep the PE busy through the
            # DMA prologue so the clock gate opens (1.2 -> 2.4 GHz) before the
            # real matmuls start.
            warm_src = constp.tile([128, PT], F16, name="warm_src")
            nc.vector.memset(warm_src, 0.01)
            warm_out = constp.tile([128, 1], F32, name="warm_out")
            ps_w = psA.tile([128, PT], F32, tag="psa", name="ps_warm")
            for i in range(20):
                nc.tensor.matmul(
                    ps_w[:, :], warm_src[:, 0:128], warm_src[:, :], start=True, stop=True
                )
            nc.vector.tensor_reduce(out=warm_out, in_=ps_w[:, 0:8], axis=AXX, op=ALU_MAX)

            # ---------------- emission helpers ----------------
            def emit_A(t):
                """L1+L2 for tile t; stores f2 (fp16) and its per-tile max."""
                x_sb = xp.tile([3, PT], F16, tag="x", name=f"x_{t}")
                nc.sync.dma_start(out=x_sb, in_=xTr[:, t * PT : (t + 1) * PT])
                ps1 = psA.tile([128, PT], F32, tag="psa", name=f"ps1_{t}")
                nc.tensor.matmul(ps1[:, :], w1_sb[:, :], x_sb[:, :], start=True, stop=True)
                h1_sb = h1p.tile([128, PT], F16, tag="h1", name=f"h1_{t}")
                nc.scalar.activation(out=h1_sb, in_=ps1, func=AF.Relu, bias=b1_sb[:, 0:1])
                for c in range(2):
                    ps2 = psA.tile([128, PT], F32, tag="psa", name=f"ps2_{t}_{c}")
                    nc.tensor.matmul(ps2[:, :], w2_sb[:, c, :], h1_sb[:, :], start=True, stop=True)
                    if c == 0:
                        nc.scalar.activation(
                            out=f2_all[:, t, c, :], in_=ps2, func=AF.Relu, bias=b2_sb[:, c : c + 1]
                        )
                    else:
                        # relu(x + b) on the DVE to balance ACT/DVE load
                        nc.vector.tensor_scalar(
                            f2_all[:, t, c, :], ps2, b2_sb[:, c : c + 1], 0.0,
                            mybir.AluOpType.add, ALU_MAX,
                        )
                nc.vector.tensor_reduce(
                    out=Mx2_sb[:, :, t : t + 1], in_=f2_all[:, t, :, :], axis=AXX, op=ALU_MAX
                )

            def emit_g(s):
                """Per-slot max over the (host-bounded) range of Mx2 columns."""
                b = bounds[s]
                for c in range(2):
                    tmp = tmpp.tile([128, Tm], F32, tag="tmp", name=f"tmpg_{c}_{s}")
                    nc.vector.tensor_mul(tmp[:, :b], Mx2_sb[:, c, :b], mask_sb[:, s, :b])
                    nc.vector.tensor_reduce(
                        out=g_sb[:, c, s : s + 1], in_=tmp[:, :b], axis=AXX, op=ALU_MAX
                    )

            def emit_Uchunk(k):
                """U[:, :, kCH:kCH+w] = W3b.T @ G2_chunk + b3 (per-tile bias)."""
                c0 = k * CH
                w = min(CH, Tm - c0)
                we = w + (w % 2)  # keep matmul free dims even
                shi = s_his[k]
                for c in range(2):
                    nc.vector.tensor_scalar_mul(
                        Gacc_sb[:, c, c0 : c0 + we], mask_sb[:, 0, c0 : c0 + we], g_sb[:, c, 0:1]
                    )
                    for s in range(1, shi + 1):
                        tmp2 = tmpp.tile([128, CH + 1], F32, tag="tmp2", name=f"tmpe_{k}_{c}_{s}")
                        nc.vector.tensor_scalar_mul(
                            tmp2[:, :we], mask_sb[:, s, c0 : c0 + we], g_sb[:, c, s : s + 1]
                        )
                        nc.vector.tensor_add(
                            Gacc_sb[:, c, c0 : c0 + we], Gacc_sb[:, c, c0 : c0 + we], tmp2[:, :we]
                        )
                    nc.scalar.copy(G2_sb[:, c, c0 : c0 + we], Gacc_sb[:, c, c0 : c0 + we])
                for m in range(4):
                    psu = psA.tile([128, PT], F32, tag="psa", name=f"psu_{k}_{m}")
                    nc.tensor.matmul(
                        psu[:, :we], w3b_sb[:, 0, m, :], G2_sb[:, 0, c0 : c0 + we],
                        start=True, stop=False,
                    )
                    nc.tensor.matmul(
                        psu[:, :we], w3b_sb[:, 1, m, :], G2_sb[:, 1, c0 : c0 + we],
                        start=False, stop=True,
                    )
                    nc.scalar.activation(
                        out=U_sb[:, m, c0 : c0 + we], in_=psu[:, :we],
                        func=AF.Identity, bias=b3_sb[:, m : m + 1],
                    )

            h3_tiles = {}

            def emit_L3(t):
                h3_sb = h3p.tile([128, 4, PT], F16, tag="h3", name=f"h3_{t}")
                for m in range(4):
                    ps3 = psB3.tile([128, PT], F32, tag="ps3", name=f"ps3_{t}_{m}")
                    nc.tensor.matmul(
                        ps3[:, :], w3a_sb[:, 0, m, :], f2_all[:, t, 0, :], start=True, stop=False
                    )
                    nc.tensor.matmul(
                        ps3[:, :], w3a_sb[:, 1, m, :], f2_all[:, t, 1, :], start=False, stop=True
                    )
                    nc.scalar.activation(
                        out=h3_sb[:, m, :], in_=ps3, func=AF.Relu, bias=U_sb[:, m, t : t + 1]
                    )
                h3_tiles[t] = h3_sb

            def emit_L4(t):
                h3_sb = h3_tiles.pop(t)
                for mg in range(4):
                    # inner dim padded to a full PSUM bank (512 f32) so each
                    # m-chunk's matmul output stays within one bank
                    ps4 = psB4.tile([128, 2, 512], F32, tag="ps4", name=f"ps4_{t}_{mg}")
                    for mi in range(2):
                        m = mg * 2 + mi
                        for k in range(4):
                            nc.tensor.matmul(
                                ps4[:, mi, :PT], w4_sb[:, k, m, :], h3_sb[:, k, :],
                                start=(k == 0), stop=(k == 3),
                            )
                    nc.vector.tensor_reduce(
                        out=Mx4_sb[:, 2 * mg : 2 * mg + 2, t : t + 1], in_=ps4[:, :, :PT], axis=AXX, op=ALU_MAX
                    )

            # ---------------- interleaved pipeline ----------------
            a_next = 0
            b_next = 0
            l3_next = 0
            u_next = 0
            g_emitted = [False] * S

            def try_unlock():
                nonlocal u_next
                for s in range(S):
                    if not g_emitted[s] and a_next >= bounds[s]:
                        emit_g(s)
                        g_emitted[s] = True
                while u_next < nchunks and all(g_emitted[s] for s in range(s_his[u_next] + 1)):
                    emit_Uchunk(u_next)
                    u_next += 1

            # phase A must lead phase B by enough tiles that B's U-table
            # chunks are always unlocked when its L3s reach the PE queue
            need = [bounds[s_his[min(i + 1, Tm - 1) // CH]] for i in range(Tm)]
            LEAD = max(max(need[i] - i for i in range(Tm)) + 1, need[0])

            # the fill phase is DVE/ACT-paced with the PE at ~50% duty, which
            # re-throttles the clock gate; pad it with dummy matmuls into a
            # psB4-pool tile (idle until the first L4, released before the
            # second one needs its slot)
            ps_dummy = psB4.tile([128, 2, 512], F32, tag="ps4", name="ps_dummy")

            while b_next < Tm:
                while a_next < min(Tm, b_next + LEAD):
                    emit_A(a_next)
                    if b_next == 0:
                        for _ in range(4):
                            nc.tensor.matmul(
                                ps_dummy[:, 0, :PT], warm_src[:, 0:128], warm_src[:, :],
                                start=True, stop=True,
                            )
                    a_next += 1
                    try_unlock()
                progressed = False
                while (
                    l3_next <= min(b_next + 1, Tm - 1)
                    and l3_next // CH < u_next
                    and l3_next < a_next
                ):
                    emit_L3(l3_next)
                    l3_next += 1
                    progressed = True
                if l3_next > b_next:
                    emit_L4(b_next)
                    b_next += 1
                    progressed = True
                if not progressed:
                    if a_next < Tm:
                        emit_A(a_next)
                        a_next += 1
                        try_unlock()
                    else:
                        raise RuntimeError("pipeline deadlock")

            nc.sync.dma_start(out=mx4.ap(), in_=Mx4_sb)

    nc.finalize()
    return nc


def _prepare(x, seg_ids, B):
    """Pad + pack segments into per-core, slot-monotone macro-tile streams."""
    counts = np.bincount(seg_ids, minlength=B)
    starts = np.concatenate([[0], np.cumsum(counts)])
    seg_tiles = [(int(c) + PT - 1) // PT for c in counts]

    SLOTS = (B + N_CORES - 1) // N_CORES
    order = np.argsort(-np.asarray(seg_tiles), kind="stable")
    core_segs: list[list[int]] = [[] for _ in range(N_CORES)]
    core_load = [0] * N_CORES
    for s in order:
        cands = [c for c in range(N_CORES) if len(core_segs[c]) < SLOTS]
        c = min(cands, key=lambda i: core_load[i])
        core_segs[c].append(int(s))
        core_load[c] += seg_tiles[s]

    # local search: swap segments between cores to shave the max load
    ideal = (sum(seg_tiles) + N_CORES - 1) // N_CORES
    for _ in range(200):
        if max(core_load) <= ideal:
            break
        hi = max(range(N_CORES), key=lambda i: core_load[i])
        improved = False
        for lo in sorted(range(N_CORES), key=lambda i: core_load[i]):
            if lo == hi:
                continue
            for ia, sa in enumerate(core_segs[hi]):
                for ib, sb in enumerate(core_segs[lo]):
                    d = seg_tiles[sa] - seg_tiles[sb]
                    if d > 0 and max(core_load[hi] - d, core_load[lo] + d) < max(
                        core_load[hi], core_load[lo]
                    ):
                        core_segs[hi][ia], core_segs[lo][ib] = sb, sa
                        core_load[hi] -= d
                        core_load[lo] += d
                        improved = True
                        break
                if improved:
                    break
            if improved:
                break
        if not improved:
            break
    Tm = max(core_load)

    # order each core's slots so cumulative tile counts track uniform
    # targets across cores => tight static pipeline bounds (small LEAD)
    from itertools import permutations

    targets = [Tm * (i + 1) / SLOTS for i in range(SLOTS)]
    for c in range(N_CORES):
        best, best_score = None, None
        for perm in permutations(core_segs[c]):
            cs, score = 0, 0.0
            for i, s in enumerate(perm):
                cs += seg_tiles[s]
                score += abs(cs - targets[i])
            if best_score is None or score < best_score:
                best, best_score = perm, score
        core_segs[c] = list(best)

    xT_cores, mask_cores, post = [], [], []
    sots = []
    for c in range(N_CORES):
        pts_list, slot_of_tile = [], []
        for slot, s in enumerate(core_segs[c]):
            seg_pts = x[starts[s] : starts[s + 1]]
            ntile = seg_tiles[s]
            padn = ntile * PT - len(seg_pts)
            if padn:
                seg_pts = np.concatenate([seg_pts, seg_pts[:padn]])
            pts_list.append(seg_pts)
            slot_of_tile += [slot] * ntile
        extra = Tm - core_load[c]
        if extra:
            # core-equalization pad tiles duplicate the LAST slot's points so
            # the tile order stays slot-monotone
            pts_list.append(np.tile(pts_list[-1][:PT], (extra, 1)))
            slot_of_tile += [SLOTS - 1] * extra
        xc = np.concatenate(pts_list).astype(np.float16)
        xT_cores.append(np.ascontiguousarray(xc.T))
        sot = np.asarray(slot_of_tile)
        sots.append(sot)
        Tmp = Tm + (Tm % 2)
        m01 = np.zeros((SLOTS, Tmp), np.float32)
        m01[:, :Tm] = sot[None, :] == np.arange(SLOTS)[:, None]
        mask_cores.append(np.ascontiguousarray(np.broadcast_to(m01[None], (128, SLOTS, Tmp))))
        post.append((core_segs[c], sot))

    # static pipeline bounds (shared across cores)
    bounds = tuple(
        int(max(np.flatnonzero(sot == s).max() for sot in sots)) + 1 for s in range(SLOTS)
    )
    nchunks = (Tm + CH - 1) // CH
    s_his = tuple(
        int(max(sot[k * CH : min((k + 1) * CH, Tm)].max() for sot in sots))
        for k in range(nchunks)
    )
    return Tm, SLOTS, xT_cores, mask_cores, post, bounds, s_his


def make_in_maps(inputs):
    """Fold BN, pack points, and build the per-core SPMD input dicts.

    Returns (key, in_maps, post, b4f) where key indexes _PROGRAM_CACHE.
    """
    x = np.asarray(inputs["x"], np.float32)
    seg_ids = np.asarray(inputs["seg_ids"])
    B = int(inputs["num_segments"])

    Wf, bf = [], []
    for i in (1, 2, 3, 4):
        W = np.asarray(inputs[f"W{i}"], np.float32)
        b = np.asarray(inputs[f"b{i}"], np.float32)
        ga = np.asarray(inputs[f"g{i}"], np.float32)
        be = np.asarray(inputs[f"be{i}"], np.float32)
        m = np.asarray(inputs[f"m{i}"], np.float32)
        v = np.asarray(inputs[f"v{i}"], np.float32)
        sc = ga / np.sqrt(v + EPS)
        Wf.append(np.ascontiguousarray(W * sc[None, :]))
        bf.append((b - m) * sc + be)
    W1f, W2f, W3f, W4f = Wf
    b1f, b2f, b3f, b4f = bf

    Tm, SLOTS, xT_cores, mask_cores, post, bounds, s_his = _prepare(x, seg_ids, B)

    w1d = W1f.astype(np.float16)
    w2d = np.ascontiguousarray(W2f.reshape(128, 2, 128).astype(np.float16))
    w3ad = np.ascontiguousarray(W3f[:256].reshape(2, 128, 4, 128).transpose(1, 0, 2, 3).astype(np.float16))
    w3bd = np.ascontiguousarray(W3f[256:].reshape(2, 128, 4, 128).transpose(1, 0, 2, 3).astype(np.float16))
    w4d = np.ascontiguousarray(W4f.reshape(4, 128, 8, 128).transpose(1, 0, 2, 3).astype(np.float16))
    b1d = np.ascontiguousarray(b1f.reshape(128, 1))
    b2d = np.ascontiguousarray(b2f.reshape(2, 128).T)
    b3d = np.ascontiguousarray(b3f.reshape(4, 128).T)

    in_maps = [
        {
            "xT": xT_cores[c],
            "mask": mask_cores[c],
            "w1": w1d,
            "w2": w2d,
            "w3a": w3ad,
            "w3b": w3bd,
            "w4": w4d,
            "b1": b1d,
            "b2": b2d,
            "b3": b3d,
        }
        for c in range(N_CORES)
    ]
    return (Tm, SLOTS, bounds, s_his), in_maps, post, b4f


def postprocess(results, post, b4f, B):
    out = np.zeros((B, 1024), np.float32)
    for c in range(N_CORES):
        mx4 = results[c]["mx4"]  # [128, 8, Tm]
        segs, sot = post[c]
        for slot, s in enumerate(segs):
            cols = np.flatnonzero(sot == slot)
            raw = mx4[:, :, cols].max(axis=2)  # [128, 8]
            out[s] = np.maximum(raw.T.reshape(1024) + b4f, 0.0)
    return out


def get_program(key):
    if key not in _PROGRAM_CACHE:
        _PROGRAM_CACHE[key] = _build_program(*key)
    return _PROGRAM_CACHE[key]


def kernel(**inputs) -> np.ndarray:
    B = int(inputs["num_segments"])
    key, in_maps, post, b4f = make_in_maps(inputs)
    nc = get_program(key)
    last_err = None
    for _ in range(3):  # retry transient NRT device wedges
        try:
            res = run_bass_kernel_spmd(nc, in_maps, core_ids=list(range(N_CORES)))
            return postprocess(res.results, post, b4f, B)
        except Exception as e:  # noqa: BLE001
            last_err = e
    raise last_err
